# revision 18
# baseline (speedup 1.0000x reference)
"""Trainium2 Bass kernel for CustomMultiHeadAttention.

Problem: T=S=1024, B=8, C=1024, H=16 heads, head_dim=64, fp32.
  q = (query @ Wq.T + bq) * scale ; k = key @ Wk.T + bk ; v = value @ Wv.T + bv
  scores = q @ k.T per (b, h); softmax over s (with key_padding_mask);
  out = (attn @ v) @ Wo.T + bo

Sharding: batch-parallel - core b owns batch element b (8 cores, SPMD, no
collectives; projection weights replicated).

v3 schedule = baseline attention inner loop (proven on HW) plus:
  - Input DMAs priority-ordered (wq,xq -> wk,xk -> wv,xv -> wo) so q-proj
    starts ~11us in (round-robin loading stalled the PE until ~45us).
  - Dummy matmuls warm the PE HAM clock gate during the DMA-only window.
  - Deferred projection queue is group-atomic (a psum accumulation group
    is never split across drain points) with ensure() need-guards.
  - Out-projection evacuates PSUM via the Scalar engine (idle there); the
    final out DMAs overlap the remaining groups.
"""

import numpy as np

import concourse.bass as bass
import concourse.tile as tile
from concourse import bacc, mybir
from concourse.bass_utils import run_bass_kernel_spmd

F32 = mybir.dt.float32
BF16 = mybir.dt.bfloat16

T = 1024
S = 1024
B = 8
C = 1024
H = 16
HD = 64
SCALE = float(HD) ** -0.5

N_CORES = 8

DEBUG_DUMPS = False


def _build(bq_any: bool, bk_any: bool, bo_any: bool):
    nc = bacc.Bacc(
        "TRN2",
        target_bir_lowering=False,
        debug=False,
        num_devices=N_CORES,
    )

    xq_d = nc.dram_tensor("xq_t", [C, T], BF16, kind="ExternalInput")
    xk_d = nc.dram_tensor("xk_t", [C, S], BF16, kind="ExternalInput")
    xv_d = nc.dram_tensor("xv_t", [C, S], BF16, kind="ExternalInput")
    wq_d = nc.dram_tensor("wq_t", [C, C], BF16, kind="ExternalInput")
    wk_d = nc.dram_tensor("wk_t", [C, C], BF16, kind="ExternalInput")
    wv_d = nc.dram_tensor("wv_t", [C, C], BF16, kind="ExternalInput")
    wo_d = nc.dram_tensor("wo_t", [C, C], BF16, kind="ExternalInput")
    bq_d = nc.dram_tensor("bq_c", [128, 8], F32, kind="ExternalInput")
    bk_d = nc.dram_tensor("bk_c", [128, 8], F32, kind="ExternalInput")
    bo_d = nc.dram_tensor("bo_r", [1, C], BF16, kind="ExternalInput")
    mb_d = nc.dram_tensor("maskb", [128, 8], F32, kind="ExternalInput")
    out_d = nc.dram_tensor("out", [T, C], F32, kind="ExternalOutput")
    z2_d = nc.dram_tensor("zscratch2", [H, T], F32, kind="Internal")
    dbg = {}
    if DEBUG_DUMPS:
        dbg["v"] = nc.dram_tensor("dbg_v", [128, 8 * H * 65], BF16,
                                  kind="ExternalOutput")
        dbg["at0"] = nc.dram_tensor("dbg_at0", [128, T], BF16,
                                    kind="ExternalOutput")

    Exp = mybir.ActivationFunctionType.Exp

    with tile.TileContext(nc) as tc:
        with (
            tc.tile_pool(name="singles", bufs=1) as singles,
            tc.tile_pool(name="wx", bufs=1) as wx,
            tc.tile_pool(name="acts", bufs=1) as acts,
            tc.tile_pool(name="stream", bufs=3) as stream,
            tc.tile_pool(name="ps0", bufs=1, space="PSUM") as ps0,
        ):
            # ---- tiny constants ----
            maskb = singles.tile([128, 8], F32)
            nc.gpsimd.dma_start(maskb, mb_d.ap())
            bq_sb = bk_sb = bo_sb = ones1 = None
            if bq_any:
                bq_sb = singles.tile([128, 8], F32)
                nc.gpsimd.dma_start(bq_sb, bq_d.ap())
            if bk_any:
                bk_sb = singles.tile([128, 8], F32)
                nc.gpsimd.dma_start(bk_sb, bk_d.ap())
            if bo_any:
                ones1 = singles.tile([1, 128], BF16)
                nc.vector.memset(ones1, 1.0)
                bo_sb = singles.tile([1, C], BF16)
                nc.gpsimd.dma_start(bo_sb, bo_d.ap())

            # ---- PE warm-up during the DMA-only window ----
            junk = singles.tile([128, 512], BF16)
            nc.vector.memset(junk, 0.0)
            for w in range(8):
                wp = ps0.tile([128, 512], F32, tag="pp", bufs=2, name=f"wm{w}")
                nc.tensor.matmul(wp, junk[:, 0:128], junk, start=True, stop=True)

            # ---- bulk input loads, priority-ordered ----
            def load8(d, tag, eng):
                ts = []
                for k in range(8):
                    t = wx.tile([128, C], BF16, tag=tag, bufs=8, name=f"{tag}{k}")
                    eng.dma_start(t, d.ap()[k * 128 : (k + 1) * 128, :])
                    ts.append(t)
                return ts

            wq_sb = load8(wq_d, "wq", nc.sync)
            xq = load8(xq_d, "xq", nc.gpsimd)
            wk_sb = load8(wk_d, "wk", nc.sync)
            xk = load8(xk_d, "xk", nc.gpsimd)
            wv_sb = load8(wv_d, "wv", nc.sync)
            xv = load8(xv_d, "xv", nc.gpsimd)
            wo_sb = load8(wo_d, "wo", nc.sync)

            # ---- persistent activations ----
            qT = [acts.tile([128, T], BF16, tag="qa", bufs=8, name=f"qT{j}")
                  for j in range(8)]
            kT = [acts.tile([128, S], BF16, tag="kt", bufs=8, name=f"kT{j}")
                  for j in range(8)]
            attnT = [acts.tile([128, T], BF16, tag="at", bufs=8, name=f"attnT{j}")
                     for j in range(8)]
            v_sb = acts.tile([128, 8, H, 65], BF16, tag="v", bufs=1)
            nc.vector.memset(v_sb[:, :, :, 64:65], 1.0)

            # ---- projection emitters (whole-group, never split) ----
            _uid = [0]

            def emit_qk_one(w_sb, x, b_sb, b_any, out, j, tci):
                tsl = slice(tci * 512, (tci + 1) * 512)
                _uid[0] += 1
                t = ps0.tile([128, 512], F32, tag="pp", bufs=2,
                             name=f"pq{_uid[0]}")
                for k in range(8):
                    nc.tensor.matmul(
                        t,
                        w_sb[k][:, j * 128 : (j + 1) * 128],
                        x[k][:, tsl],
                        start=(k == 0),
                        stop=(k == 7),
                    )
                if b_any:
                    nc.vector.tensor_scalar_add(out[:, tsl], t, b_sb[:, j : j + 1])
                else:
                    nc.vector.tensor_copy(out[:, tsl], t)

            def emit_v_one(s, oc):
                t = ps0.tile([128, 512], F32, tag="pp", bufs=2, name=f"pv{s}_{oc}")
                for k in range(8):
                    nc.tensor.matmul(
                        t,
                        xv[k][:, s * 128 : (s + 1) * 128],
                        wv_sb[k][:, oc * 512 : (oc + 1) * 512],
                        start=(k == 0),
                        stop=(k == 7),
                    )
                nc.vector.tensor_copy(
                    v_sb[:, s, 8 * oc : 8 * oc + 8, 0:64],
                    t.rearrange("p (h d) -> p h d", d=64),
                )

            # upfront: q-proj j0..3 (needs only wq+xq), then k j0, then the
            # first v slices for head-0 AV.
            for j in range(4):
                for tci in range(2):
                    emit_qk_one(wq_sb, xq, bq_sb, bq_any, qT[j], j, tci)
            for tci in range(2):
                emit_qk_one(wk_sb, xk, bk_sb, bk_any, kT[0], 0, tci)
            for s in range(3):
                emit_v_one(s, 0)

            # ---- deferred queue of whole groups, need-ordered ----
            from collections import deque

            pending = deque()  # (key, emit_fn)
            done_keys = set()

            def q_push(key, fn):
                pending.append((key, fn))

            budget = [0]

            def drain(n):
                # n is in matmul units; groups are 8 MMs and atomic.
                budget[0] += n
                while budget[0] >= 8 and pending:
                    key, fn = pending.popleft()
                    fn()
                    budget[0] -= 8
                    if not pending or pending[0][0] != key:
                        done_keys.add(key)

            def ensure(key):
                while key not in done_keys and pending:
                    k2, fn = pending.popleft()
                    fn()
                    if not pending or pending[0][0] != k2:
                        done_keys.add(k2)

            def mk_qk(which, jj, tt):
                if which == "q":
                    return lambda: emit_qk_one(wq_sb, xq, bq_sb, bq_any,
                                               qT[jj], jj, tt)
                return lambda: emit_qk_one(wk_sb, xk, bk_sb, bk_any,
                                           kT[jj], jj, tt)

            def mk_v(ss, oo):
                return lambda: emit_v_one(ss, oo)

            # need-order: q j4..7 (ready at ~11us), k j1 (pair 1), v-oc0
            # tail (head 0 epilogue on), k j2, v-oc1 (heads 8+), k j3..7.
            for j in (4, 5, 6, 7):
                for tci in range(2):
                    q_push(("qk", j), mk_qk("q", j, tci))
            for tci in range(2):
                q_push(("qk", 1), mk_qk("k", 1, tci))
            for s in range(3, 8):
                q_push(("v", 0, s), mk_v(s, 0))
            for tci in range(2):
                q_push(("qk", 2), mk_qk("k", 2, tci))
            for s in range(8):
                q_push(("v", 1, s), mk_v(s, 1))
            for j in (3, 4, 5, 6, 7):
                for tci in range(2):
                    q_push(("qk", j), mk_qk("k", j, tci))

            # ---------------- attention, one head at a time (baseline) ----
            # PSUM: sc [128, T] x2 bufs (4 banks) + av [65, T] (2 banks)
            # + pp (2 banks) = 8.
            psB_cm = tc.tile_pool(name="psB", bufs=1, space="PSUM")
            psB = psB_cm.__enter__()
            for h in range(H):
                j, half = h // 2, h % 2
                if half == 0 and j >= 1:
                    ensure(("qk", j))
                av = psB.tile([65, T], F32, tag="av", bufs=1, name=f"av{h}")
                es = [None] * 8
                for s in range(8):
                    sc = psB.tile([128, T], F32, tag="sc", bufs=2,
                                  name=f"sc{h}_{s}")
                    ksl = kT[j][64 * half : 64 * half + 64,
                                s * 128 : (s + 1) * 128]
                    nc.tensor.matmul(
                        sc[:, 0:512], ksl,
                        qT[j][64 * half : 64 * half + 64, 0:512],
                        start=True, stop=True,
                    )
                    nc.tensor.matmul(
                        sc[:, 512:1024], ksl,
                        qT[j][64 * half : 64 * half + 64, 512:1024],
                        start=True, stop=True,
                    )
                    e = stream.tile([128, T], BF16, tag="e", bufs=3,
                                    name=f"e{h}_{s}")
                    nc.scalar.activation(
                        e, sc, Exp, bias=maskb[:, s : s + 1], scale=SCALE
                    )
                    es[s] = e
                    if s >= 1:
                        if s - 1 >= 3 or h >= 8:
                            ensure(("v", h // 8, s - 1))
                        for tcn in range(2):
                            tsl = slice(tcn * 512, (tcn + 1) * 512)
                            nc.tensor.matmul(
                                av[:, tsl],
                                v_sb[:, s - 1, h, :],
                                es[s - 1][:, tsl],
                                start=(s == 1),
                                stop=False,
                            )
                    drain(6 if h == 0 else 3)
                ensure(("v", h // 8, 7))
                for tcn in range(2):
                    tsl = slice(tcn * 512, (tcn + 1) * 512)
                    nc.tensor.matmul(
                        av[:, tsl], v_sb[:, 7, h, :], es[7][:, tsl],
                        start=False, stop=True,
                    )
                # epilogue (baseline): [num; Z] to SBUF; Z row hops to
                # partition 0 via sbuf->sbuf DMA; reciprocal; DRAM broadcast
                # of 1/Z across 64 partitions; normalize.
                if half == 0:
                    nt = attnT[j]  # row-64 scratch overwritten by odd head
                else:
                    nt = stream.tile([65, T], BF16, tag="nt", bufs=2,
                                     name=f"nt{h}")
                nc.vector.tensor_copy(nt[0:65, :], av[0:65, :])
                zrb = stream.tile([1, T], BF16, tag="zrb", bufs=1,
                                  name=f"zrb{h}")
                nc.gpsimd.dma_start(zrb, nt[64:65, :])
                zr = stream.tile([1, T], F32, tag="zr", bufs=1, name=f"zr{h}")
                nc.vector.tensor_copy(zr, zrb)
                nc.vector.reciprocal_approx_fast(out=zr, in_=zr)
                nc.sync.dma_start(z2_d.ap()[h : h + 1, :], zr)
                zbc = stream.tile([64, T], F32, tag="zbc", bufs=2,
                                  name=f"zbc{h}")
                nc.sync.dma_start(
                    zbc, z2_d.ap()[h : h + 1, :].to_broadcast((64, T))
                )
                if half == 0:
                    nc.vector.tensor_mul(
                        attnT[j][0:64, :], attnT[j][0:64, :], zbc
                    )
                else:
                    nc.vector.tensor_mul(nt[0:64, :], nt[0:64, :], zbc)
                    nc.sync.dma_start(attnT[j][64:128, :], nt[0:64, :])
                drain(4)
            drain(10 ** 9)
            if DEBUG_DUMPS:
                nc.sync.dma_start(
                    dbg["v"].ap(), v_sb.rearrange("p a b c -> p (a b c)")
                )
                nc.sync.dma_start(dbg["at0"].ap(), attnT[0][:, :])
            psB_cm.__exit__(None, None, None)

            # ---------------- output projection ----------------
            # Baseline structure (3 tt-groups in flight); Scalar engine
            # evacuates PSUM, sync DMA writes out.
            with tc.tile_pool(name="psC", bufs=1, space="PSUM") as psC:
                groups = [(tt, oc) for tt in range(8) for oc in range(2)]
                pso = {}

                def g_mms(tt, oc, its):
                    for it in its:
                        nc.tensor.matmul(
                            pso[(tt, oc)],
                            attnT[it][:, tt * 128 : (tt + 1) * 128],
                            wo_sb[it][:, oc * 512 : (oc + 1) * 512],
                            start=(it == 0),
                            stop=(it == 7 and not bo_any),
                        )

                def g_finish(tt, oc):
                    if bo_any:
                        nc.tensor.matmul(
                            pso[(tt, oc)],
                            ones1[0:1, 0:128],
                            bo_sb[0:1, oc * 512 : (oc + 1) * 512],
                            start=False, stop=True,
                        )
                    osb = stream.tile([128, 512], F32, tag="osb", bufs=3,
                                      name=f"osb{tt}_{oc}")
                    nc.scalar.copy(osb, pso[(tt, oc)])
                    nc.sync.dma_start(
                        out_d.ap()[
                            tt * 128 : (tt + 1) * 128,
                            oc * 512 : (oc + 1) * 512,
                        ],
                        osb,
                    )

                # partials (it=0..6) first so the it=7 waits (on the last
                # head's normalize) overlap other groups' matmuls.
                inflight = []
                gi = 0
                while gi < len(groups) or inflight:
                    while gi < len(groups) and len(inflight) < 6:
                        tt, oc = groups[gi]
                        pso[(tt, oc)] = psC.tile(
                            [128, 512], F32, tag="pc", bufs=6,
                            name=f"pso{tt}_{oc}",
                        )
                        g_mms(tt, oc, range(7))
                        inflight.append((tt, oc))
                        gi += 1
                    tt, oc = inflight.pop(0)
                    g_mms(tt, oc, [7])
                    g_finish(tt, oc)

    nc.compile()
    return nc


_last_results = None


def kernel(
    query,
    key,
    value,
    key_padding_mask,
    Wq,
    bq,
    Wk,
    bk,
    Wv,
    bv,
    Wo,
    bo,
    _trace=False,
):
    global _last_results
    query = np.asarray(query, np.float32)
    key = np.asarray(key, np.float32)
    value = np.asarray(value, np.float32)
    mask = np.asarray(key_padding_mask, bool)
    Wq = np.asarray(Wq, np.float32)
    Wk = np.asarray(Wk, np.float32)
    Wv = np.asarray(Wv, np.float32)
    Wo = np.asarray(Wo, np.float32)
    bq = np.asarray(bq, np.float32)
    bk = np.asarray(bk, np.float32)
    bv = np.asarray(bv, np.float32)
    bo = np.asarray(bo, np.float32)

    # v-bias folds into the output bias: softmax rows sum to 1, so
    # attn @ (v + bv) = attn @ v + bv, and (x + bv) @ Wo.T = x@Wo.T + Wo@bv.
    bo_eff = bo + Wo @ bv

    nc = _build(
        bq_any=bool(bq.any()),
        bk_any=bool(bk.any()),
        bo_any=bool(bo_eff.any()),
    )

    import ml_dtypes

    bf16 = ml_dtypes.bfloat16
    wqT = np.ascontiguousarray(Wq.T).astype(bf16)
    wkT = np.ascontiguousarray(Wk.T).astype(bf16)
    wvT = np.ascontiguousarray(Wv.T).astype(bf16)
    woT = np.ascontiguousarray(Wo.T).astype(bf16)
    bq_c = np.ascontiguousarray(bq.reshape(8, 128).T)
    bk_c = np.ascontiguousarray(bk.reshape(8, 128).T)
    bo_r = bo_eff.reshape(1, C)

    in_maps = []
    for b in range(N_CORES):
        maskbias = np.where(mask[b], np.float32(-1e30), np.float32(0.0)).astype(
            np.float32
        )
        in_maps.append(
            {
                "xq_t": np.ascontiguousarray(query[:, b, :].T).astype(bf16),
                "xk_t": np.ascontiguousarray(key[:, b, :].T).astype(bf16),
                "xv_t": np.ascontiguousarray(value[:, b, :].T).astype(bf16),
                "wq_t": wqT,
                "wk_t": wkT,
                "wv_t": wvT,
                "wo_t": woT,
                "bq_c": bq_c,
                "bk_c": bk_c,
                "bo_r": bo_r.astype(bf16),
                "maskb": np.ascontiguousarray(maskbias.reshape(8, 128).T),
            }
        )

    res = run_bass_kernel_spmd(
        nc,
        in_maps,
        core_ids=list(range(N_CORES)),
        trace=_trace,
    )
    _last_results = res
    out = np.stack([res.results[b]["out"] for b in range(N_CORES)], axis=1)
    return out.astype(np.float32)


# revision 19
# speedup vs baseline: 1.0070x; 1.0070x over previous
"""Trainium2 Bass kernel for CustomMultiHeadAttention.

Problem: T=S=1024, B=8, C=1024, H=16 heads, head_dim=64, fp32.
  q = (query @ Wq.T + bq) * scale ; k = key @ Wk.T + bk ; v = value @ Wv.T + bv
  scores = q @ k.T per (b, h); softmax over s (with key_padding_mask);
  out = (attn @ v) @ Wo.T + bo

Sharding: batch-parallel - core b owns batch element b (8 cores, SPMD, no
collectives; projection weights replicated).

v3 schedule = baseline attention inner loop (proven on HW) plus:
  - Input DMAs priority-ordered (wq,xq -> wk,xk -> wv,xv -> wo) so q-proj
    starts ~11us in (round-robin loading stalled the PE until ~45us).
  - Dummy matmuls warm the PE HAM clock gate during the DMA-only window.
  - Deferred projection queue is group-atomic (a psum accumulation group
    is never split across drain points) with ensure() need-guards.
  - Out-projection evacuates PSUM via the Scalar engine (idle there); the
    final out DMAs overlap the remaining groups.
"""

import numpy as np

import concourse.bass as bass
import concourse.tile as tile
from concourse import bacc, mybir
from concourse.bass_utils import run_bass_kernel_spmd

F32 = mybir.dt.float32
BF16 = mybir.dt.bfloat16

T = 1024
S = 1024
B = 8
C = 1024
H = 16
HD = 64
SCALE = float(HD) ** -0.5

N_CORES = 8

DEBUG_DUMPS = False


def _build(bq_any: bool, bk_any: bool, bo_any: bool):
    nc = bacc.Bacc(
        "TRN2",
        target_bir_lowering=False,
        debug=False,
        num_devices=N_CORES,
    )

    xq_d = nc.dram_tensor("xq_t", [C, T], BF16, kind="ExternalInput")
    xk_d = nc.dram_tensor("xk_t", [C, S], BF16, kind="ExternalInput")
    xv_d = nc.dram_tensor("xv_t", [C, S], BF16, kind="ExternalInput")
    wq_d = nc.dram_tensor("wq_t", [C, C], BF16, kind="ExternalInput")
    wk_d = nc.dram_tensor("wk_t", [C, C], BF16, kind="ExternalInput")
    wv_d = nc.dram_tensor("wv_t", [C, C], BF16, kind="ExternalInput")
    wo_d = nc.dram_tensor("wo_t", [C, C], BF16, kind="ExternalInput")
    bq_d = nc.dram_tensor("bq_c", [128, 8], F32, kind="ExternalInput")
    bk_d = nc.dram_tensor("bk_c", [128, 8], F32, kind="ExternalInput")
    bo_d = nc.dram_tensor("bo_r", [1, C], BF16, kind="ExternalInput")
    mb_d = nc.dram_tensor("maskb", [128, 8], F32, kind="ExternalInput")
    out_d = nc.dram_tensor("out", [T, C], F32, kind="ExternalOutput")
    z2_d = nc.dram_tensor("zscratch2", [H, T], F32, kind="Internal")
    dbg = {}
    if DEBUG_DUMPS:
        dbg["v"] = nc.dram_tensor("dbg_v", [128, 8 * H * 65], BF16,
                                  kind="ExternalOutput")
        dbg["at0"] = nc.dram_tensor("dbg_at0", [128, T], BF16,
                                    kind="ExternalOutput")

    Exp = mybir.ActivationFunctionType.Exp

    with tile.TileContext(nc) as tc:
        with (
            tc.tile_pool(name="singles", bufs=1) as singles,
            tc.tile_pool(name="wx", bufs=1) as wx,
            tc.tile_pool(name="acts", bufs=1) as acts,
            tc.tile_pool(name="stream", bufs=3) as stream,
            tc.tile_pool(name="ps0", bufs=1, space="PSUM") as ps0,
        ):
            # ---- tiny constants ----
            maskb = singles.tile([128, 8], F32)
            nc.gpsimd.dma_start(maskb, mb_d.ap())
            bq_sb = bk_sb = bo_sb = ones1 = None
            if bq_any:
                bq_sb = singles.tile([128, 8], F32)
                nc.gpsimd.dma_start(bq_sb, bq_d.ap())
            if bk_any:
                bk_sb = singles.tile([128, 8], F32)
                nc.gpsimd.dma_start(bk_sb, bk_d.ap())
            if bo_any:
                ones1 = singles.tile([1, 128], BF16)
                nc.vector.memset(ones1, 1.0)
                bo_sb = singles.tile([1, C], BF16)
                nc.gpsimd.dma_start(bo_sb, bo_d.ap())

            # ---- PE warm-up during the DMA-only window ----
            junk = singles.tile([128, 512], BF16)
            nc.vector.memset(junk, 0.0)
            for w in range(8):
                wp = ps0.tile([128, 512], F32, tag="pp", bufs=2, name=f"wm{w}")
                nc.tensor.matmul(wp, junk[:, 0:128], junk, start=True, stop=True)

            # ---- bulk input loads, priority-ordered ----
            def load8(d, tag, eng):
                ts = []
                for k in range(8):
                    t = wx.tile([128, C], BF16, tag=tag, bufs=8, name=f"{tag}{k}")
                    eng.dma_start(t, d.ap()[k * 128 : (k + 1) * 128, :])
                    ts.append(t)
                return ts

            wq_sb = load8(wq_d, "wq", nc.sync)
            xq = load8(xq_d, "xq", nc.gpsimd)
            wk_sb = load8(wk_d, "wk", nc.sync)
            xk = load8(xk_d, "xk", nc.gpsimd)
            wv_sb = load8(wv_d, "wv", nc.sync)
            xv = load8(xv_d, "xv", nc.gpsimd)
            wo_sb = load8(wo_d, "wo", nc.sync)

            # ---- persistent activations ----
            qT = [acts.tile([128, T], BF16, tag="qa", bufs=8, name=f"qT{j}")
                  for j in range(8)]
            kT = [acts.tile([128, S], BF16, tag="kt", bufs=8, name=f"kT{j}")
                  for j in range(8)]
            attnT = [acts.tile([128, T], BF16, tag="at", bufs=8, name=f"attnT{j}")
                     for j in range(8)]
            v_sb = acts.tile([128, 8, H, 65], BF16, tag="v", bufs=1)
            nc.vector.memset(v_sb[:, :, :, 64:65], 1.0)

            # ---- projection emitters (whole-group, never split) ----
            _uid = [0]

            def emit_qk_one(w_sb, x, b_sb, b_any, out, j, tci):
                tsl = slice(tci * 512, (tci + 1) * 512)
                _uid[0] += 1
                t = ps0.tile([128, 512], F32, tag="pp", bufs=2,
                             name=f"pq{_uid[0]}")
                for k in range(8):
                    nc.tensor.matmul(
                        t,
                        w_sb[k][:, j * 128 : (j + 1) * 128],
                        x[k][:, tsl],
                        start=(k == 0),
                        stop=(k == 7),
                    )
                if b_any:
                    nc.vector.tensor_scalar_add(out[:, tsl], t, b_sb[:, j : j + 1])
                else:
                    nc.vector.tensor_copy(out[:, tsl], t)

            def emit_v_one(s, oc):
                t = ps0.tile([128, 512], F32, tag="pp", bufs=2, name=f"pv{s}_{oc}")
                for k in range(8):
                    nc.tensor.matmul(
                        t,
                        xv[k][:, s * 128 : (s + 1) * 128],
                        wv_sb[k][:, oc * 512 : (oc + 1) * 512],
                        start=(k == 0),
                        stop=(k == 7),
                    )
                nc.vector.tensor_copy(
                    v_sb[:, s, 8 * oc : 8 * oc + 8, 0:64],
                    t.rearrange("p (h d) -> p h d", d=64),
                )

            # upfront: q-proj j0..3 (needs only wq+xq), then k j0, then the
            # first v slices for head-0 AV.
            for j in range(4):
                for tci in range(2):
                    emit_qk_one(wq_sb, xq, bq_sb, bq_any, qT[j], j, tci)
            for tci in range(2):
                emit_qk_one(wk_sb, xk, bk_sb, bk_any, kT[0], 0, tci)
            for s in range(3):
                emit_v_one(s, 0)

            # ---- deferred queue of whole groups, need-ordered ----
            from collections import deque

            pending = deque()  # (key, emit_fn)
            done_keys = set()

            def q_push(key, fn):
                pending.append((key, fn))

            budget = [0]

            def drain(n):
                # n is in matmul units; groups are 8 MMs and atomic.
                budget[0] += n
                while budget[0] >= 8 and pending:
                    key, fn = pending.popleft()
                    fn()
                    budget[0] -= 8
                    if not pending or pending[0][0] != key:
                        done_keys.add(key)

            def ensure(key):
                while key not in done_keys and pending:
                    k2, fn = pending.popleft()
                    fn()
                    if not pending or pending[0][0] != k2:
                        done_keys.add(k2)

            def mk_qk(which, jj, tt):
                if which == "q":
                    return lambda: emit_qk_one(wq_sb, xq, bq_sb, bq_any,
                                               qT[jj], jj, tt)
                return lambda: emit_qk_one(wk_sb, xk, bk_sb, bk_any,
                                           kT[jj], jj, tt)

            def mk_v(ss, oo):
                return lambda: emit_v_one(ss, oo)

            # need-order: q j4..7 (ready at ~11us), k j1 (pair 1), v-oc0
            # tail (head 0 epilogue on), k j2, v-oc1 (heads 8+), k j3..7.
            for j in (4, 5, 6, 7):
                for tci in range(2):
                    q_push(("qk", j), mk_qk("q", j, tci))
            for tci in range(2):
                q_push(("qk", 1), mk_qk("k", 1, tci))
            for s in range(3, 8):
                q_push(("v", 0, s), mk_v(s, 0))
            for tci in range(2):
                q_push(("qk", 2), mk_qk("k", 2, tci))
            for s in range(8):
                q_push(("v", 1, s), mk_v(s, 1))
            for j in (3, 4, 5, 6, 7):
                for tci in range(2):
                    q_push(("qk", j), mk_qk("k", j, tci))

            # ---------------- attention, one head at a time (baseline) ----
            # PSUM: sc [128, T] x2 bufs (4 banks) + av [65, T] (2 banks)
            # + pp (2 banks) = 8.
            psB_cm = tc.tile_pool(name="psB", bufs=1, space="PSUM")
            psB = psB_cm.__enter__()
            for h in range(H):
                j, half = h // 2, h % 2
                if half == 0 and j >= 1:
                    ensure(("qk", j))
                av = psB.tile([65, T], F32, tag="av", bufs=1, name=f"av{h}")
                es = [None] * 8
                for s in range(8):
                    sc = psB.tile([128, T], F32, tag="sc", bufs=2,
                                  name=f"sc{h}_{s}")
                    ksl = kT[j][64 * half : 64 * half + 64,
                                s * 128 : (s + 1) * 128]
                    nc.tensor.matmul(
                        sc[:, 0:512], ksl,
                        qT[j][64 * half : 64 * half + 64, 0:512],
                        start=True, stop=True,
                    )
                    nc.tensor.matmul(
                        sc[:, 512:1024], ksl,
                        qT[j][64 * half : 64 * half + 64, 512:1024],
                        start=True, stop=True,
                    )
                    e = stream.tile([128, T], BF16, tag="e", bufs=3,
                                    name=f"e{h}_{s}")
                    nc.scalar.activation(
                        e, sc, Exp, bias=maskb[:, s : s + 1], scale=SCALE
                    )
                    es[s] = e
                    if s >= 1:
                        if s - 1 >= 3 or h >= 8:
                            ensure(("v", h // 8, s - 1))
                        for tcn in range(2):
                            tsl = slice(tcn * 512, (tcn + 1) * 512)
                            nc.tensor.matmul(
                                av[:, tsl],
                                v_sb[:, s - 1, h, :],
                                es[s - 1][:, tsl],
                                start=(s == 1),
                                stop=False,
                            )
                    drain(6 if h == 0 else 3)
                ensure(("v", h // 8, 7))
                for tcn in range(2):
                    tsl = slice(tcn * 512, (tcn + 1) * 512)
                    nc.tensor.matmul(
                        av[:, tsl], v_sb[:, 7, h, :], es[7][:, tsl],
                        start=False, stop=True,
                    )
                # epilogue (baseline): [num; Z] to SBUF; Z row hops to
                # partition 0 via sbuf->sbuf DMA; reciprocal; DRAM broadcast
                # of 1/Z across 64 partitions; normalize.
                if half == 0:
                    nt = attnT[j]  # row-64 scratch overwritten by odd head
                else:
                    nt = stream.tile([65, T], BF16, tag="nt", bufs=2,
                                     name=f"nt{h}")
                nc.vector.tensor_copy(nt[0:65, :], av[0:65, :])
                zrb = stream.tile([1, T], BF16, tag="zrb", bufs=1,
                                  name=f"zrb{h}")
                nc.gpsimd.dma_start(zrb, nt[64:65, :])
                zr = stream.tile([1, T], F32, tag="zr", bufs=1, name=f"zr{h}")
                nc.vector.tensor_copy(zr, zrb)
                nc.vector.reciprocal_approx_fast(out=zr, in_=zr)
                nc.sync.dma_start(z2_d.ap()[h : h + 1, :], zr)
                zbc = stream.tile([64, T], F32, tag="zbc", bufs=2,
                                  name=f"zbc{h}")
                nc.sync.dma_start(
                    zbc, z2_d.ap()[h : h + 1, :].to_broadcast((64, T))
                )
                if half == 0:
                    nc.vector.tensor_mul(
                        attnT[j][0:64, :], attnT[j][0:64, :], zbc
                    )
                else:
                    nc.vector.tensor_mul(nt[0:64, :], nt[0:64, :], zbc)
                    nc.sync.dma_start(attnT[j][64:128, :], nt[0:64, :])
                drain(4)
            drain(10 ** 9)
            if DEBUG_DUMPS:
                nc.sync.dma_start(
                    dbg["v"].ap(), v_sb.rearrange("p a b c -> p (a b c)")
                )
                nc.sync.dma_start(dbg["at0"].ap(), attnT[0][:, :])
            psB_cm.__exit__(None, None, None)

            # ---------------- output projection ----------------
            # Baseline structure (3 tt-groups in flight); Scalar engine
            # evacuates PSUM, sync DMA writes out.
            with tc.tile_pool(name="psC", bufs=1, space="PSUM") as psC:
                for tt in range(8):
                    pso = [
                        psC.tile([128, 512], F32, tag="pc", bufs=6,
                                 name=f"pso{tt}_{oc}")
                        for oc in range(2)
                    ]
                    for it in range(8):
                        for oc in range(2):
                            nc.tensor.matmul(
                                pso[oc],
                                attnT[it][:, tt * 128 : (tt + 1) * 128],
                                wo_sb[it][:, oc * 512 : (oc + 1) * 512],
                                start=(it == 0),
                                stop=(it == 7 and not bo_any),
                            )
                    for oc in range(2):
                        if bo_any:
                            nc.tensor.matmul(
                                pso[oc],
                                ones1[0:1, 0:128],
                                bo_sb[0:1, oc * 512 : (oc + 1) * 512],
                                start=False, stop=True,
                            )
                        osb = stream.tile([128, 512], F32, tag="osb", bufs=3,
                                          name=f"osb{tt}_{oc}")
                        nc.scalar.copy(osb, pso[oc])
                        nc.sync.dma_start(
                            out_d.ap()[
                                tt * 128 : (tt + 1) * 128,
                                oc * 512 : (oc + 1) * 512,
                            ],
                            osb,
                        )

    nc.compile()
    return nc


_last_results = None


def kernel(
    query,
    key,
    value,
    key_padding_mask,
    Wq,
    bq,
    Wk,
    bk,
    Wv,
    bv,
    Wo,
    bo,
    _trace=False,
):
    global _last_results
    query = np.asarray(query, np.float32)
    key = np.asarray(key, np.float32)
    value = np.asarray(value, np.float32)
    mask = np.asarray(key_padding_mask, bool)
    Wq = np.asarray(Wq, np.float32)
    Wk = np.asarray(Wk, np.float32)
    Wv = np.asarray(Wv, np.float32)
    Wo = np.asarray(Wo, np.float32)
    bq = np.asarray(bq, np.float32)
    bk = np.asarray(bk, np.float32)
    bv = np.asarray(bv, np.float32)
    bo = np.asarray(bo, np.float32)

    # v-bias folds into the output bias: softmax rows sum to 1, so
    # attn @ (v + bv) = attn @ v + bv, and (x + bv) @ Wo.T = x@Wo.T + Wo@bv.
    bo_eff = bo + Wo @ bv

    nc = _build(
        bq_any=bool(bq.any()),
        bk_any=bool(bk.any()),
        bo_any=bool(bo_eff.any()),
    )

    import ml_dtypes

    bf16 = ml_dtypes.bfloat16
    wqT = np.ascontiguousarray(Wq.T).astype(bf16)
    wkT = np.ascontiguousarray(Wk.T).astype(bf16)
    wvT = np.ascontiguousarray(Wv.T).astype(bf16)
    woT = np.ascontiguousarray(Wo.T).astype(bf16)
    bq_c = np.ascontiguousarray(bq.reshape(8, 128).T)
    bk_c = np.ascontiguousarray(bk.reshape(8, 128).T)
    bo_r = bo_eff.reshape(1, C)

    in_maps = []
    for b in range(N_CORES):
        maskbias = np.where(mask[b], np.float32(-1e30), np.float32(0.0)).astype(
            np.float32
        )
        in_maps.append(
            {
                "xq_t": np.ascontiguousarray(query[:, b, :].T).astype(bf16),
                "xk_t": np.ascontiguousarray(key[:, b, :].T).astype(bf16),
                "xv_t": np.ascontiguousarray(value[:, b, :].T).astype(bf16),
                "wq_t": wqT,
                "wk_t": wkT,
                "wv_t": wvT,
                "wo_t": woT,
                "bq_c": bq_c,
                "bk_c": bk_c,
                "bo_r": bo_r.astype(bf16),
                "maskb": np.ascontiguousarray(maskbias.reshape(8, 128).T),
            }
        )

    res = run_bass_kernel_spmd(
        nc,
        in_maps,
        core_ids=list(range(N_CORES)),
        trace=_trace,
    )
    _last_results = res
    out = np.stack([res.results[b]["out"] for b in range(N_CORES)], axis=1)
    return out.astype(np.float32)
